# revision 24
# baseline (speedup 1.0000x reference)
"""Trainium2 Bass kernel for nn_DynamicGraphNet (2-layer GNN attention message passing).

Contract: kernel(**inputs) takes the FULL unsharded inputs (as produced by the
reference's setup_inputs) and returns the full output (output_vec[64], x[2176, 64]).

Strategy: the reference's edge_index is a deterministic complete-bipartite
block graph (64 input -> 2048 hidden -> 64 output).  We verify that structure
at runtime; when it holds, the gather/scatter collapses into dense per-head
matmuls.  The whole problem is tiny enough to live in one NeuronCore's SBUF,
and on-chip collectives cost ~200us fixed each, so the fastest layout is a
single-core kernel with no cross-core communication (measured ~2x faster
end-to-end than an 8-way shard paying two AllGathers per layer).  If the
structure check fails, a general jax fallback implements the exact reference
math for arbitrary graphs.

Layout notes:
- Node features are kept transposed (XT [64 feat, 2176 nodes]) with columns
  reordered to [hidden(2048) | input(64) | output(64)] so every 128-node tile
  is cleanly aligned.
- Block-1 logits (input->hidden edges) are packed [128, 4096]: partition
  p = src_i + 64*(head//2), free = (head%2)*2048 + tgt_j, so elementwise
  passes run at full 128-partition width.
- Block-2 logits (hidden->output edges) are packed [128, 4096]: partition
  p = tgt_j%128, free = head*1024 + (j//128)*64 + out_o.
- The global edge-softmax uses per-partition row maxima, folded back exactly
  via exp(rowmax - globalmax) scaling of the exp'd logits, with the global
  1/sum folded into the wo matrix's rows.  Math is exact (fp reassociation
  only).
- Hardware quirk found empirically: all matmuls accumulating into the same
  PSUM tile must use operands with the SAME base partition (mixing base 0/64
  operands within one PSUM tile's groups aborts the NEFF at runtime).
"""

import functools
import os
import sys

import numpy as np

for _p in ("/root/.axon_site", "/root/.axon_site/_ro/trn_rl_repo",
           "/root/.axon_site/_ro/pypackages", "/opt/trn_rl_repo"):
    if os.path.isdir(_p) and _p not in sys.path:
        sys.path.append(_p)

N_NODES = 2176
N_IN = 64
N_HID = 2048
N_OUT = 64
D = 64
H = 4
HD = D * H  # 256
E1 = N_IN * N_HID
E2 = N_HID * N_OUT
E = E1 + E2

# packed input blob layout (floats)
OFF_XT = 0                       # [64, 2176]
OFF_EW1 = D * N_NODES            # [64, 2048] block-1 edge weights (i-major)
OFF_EW2P = OFF_EW1 + N_IN * N_HID    # [128, 1024] block-2 packed
OFF_W = OFF_EW2P + 128 * 1024
LAYER_W = 4 * 16384 + H + D      # wq+wks+wv+wo, then we, bo
NBLOB = OFF_W + 2 * LAYER_W + D + 1


def _expected_edge_index() -> np.ndarray:
    hs, os_ = N_IN, N_IN + N_HID
    s1 = np.repeat(np.arange(0, N_IN), N_HID)
    t1 = np.tile(np.arange(hs, os_), N_IN)
    s2 = np.repeat(np.arange(hs, os_), N_OUT)
    t2 = np.tile(np.arange(os_, N_NODES), N_HID)
    return np.stack([np.concatenate([s1, s2]), np.concatenate([t1, t2])]).astype(np.int32)


# ---------------------------------------------------------------------------
# Bass program (single core)
# ---------------------------------------------------------------------------

def _build_program():
    import concourse.bacc as bacc
    import concourse.mybir as mybir
    from concourse import masks
    from concourse.tile import TileContext

    F32 = mybir.dt.float32
    F32R = mybir.dt.float32r
    AF = mybir.ActivationFunctionType
    ALU = mybir.AluOpType
    AX = mybir.AxisListType

    nc = bacc.Bacc("TRN2", target_bir_lowering=False, debug=False, num_devices=1)

    blob_d = nc.dram_tensor("blob", [1, NBLOB], F32, kind="ExternalInput")
    xout_d = nc.dram_tensor("xout", [D, N_NODES + 1], F32, kind="ExternalOutput")

    def bview(off, p, f):
        return blob_d[0:1, off:off + p * f].rearrange("a (p f) -> (a p) f", p=p)

    HIDC = slice(0, N_HID)
    IC = slice(N_HID, N_HID + N_IN)
    OC = slice(N_HID + N_IN, N_NODES)
    # N-chunks for 2176-wide matmul outputs (PSUM free <= 512)
    CH2176 = [(0, 512), (512, 512), (1024, 512), (1536, 512), (2048, 128)]
    CH2048 = [(0, 512), (512, 512), (1024, 512), (1536, 512)]

    with TileContext(nc) as tc:
        with (
            tc.tile_pool(name="consts", bufs=1) as cp,
            tc.tile_pool(name="wts", bufs=1) as wpool,
            tc.tile_pool(name="work", bufs=1) as wk,
            tc.tile_pool(name="ps", bufs=1, space="PSUM") as ps,
        ):
            # ---- constants ----
            ident = cp.tile([128, 128], F32)
            masks.make_identity(nc, ident[:])
            ones = cp.tile([1, 128], F32)
            nc.vector.memset(ones[:], 1.0)
            onesc = cp.tile([128, 1], F32)
            nc.vector.memset(onesc[:], 1.0)
            half = cp.tile([128, 2], F32)
            nc.vector.memset(half[:], 0.0)
            nc.vector.memset(half[0:64, 0:1], 1.0)
            nc.vector.memset(half[64:128, 1:2], 1.0)
            # head-expansion one-hots: expA[k, m]=1 iff m//64==k (k<2); expB k-2
            expA_cols = cp.tile([128, H], F32)
            expB_cols = cp.tile([128, H], F32)
            nc.vector.memset(expA_cols[:], 0.0)
            nc.vector.memset(expB_cols[:], 0.0)
            nc.vector.memset(expA_cols[0:64, 0:1], 1.0)
            nc.vector.memset(expA_cols[64:128, 1:2], 1.0)
            nc.vector.memset(expB_cols[0:64, 2:3], 1.0)
            nc.vector.memset(expB_cols[64:128, 3:4], 1.0)
            expA_p = ps.tile([H, 128], F32, tag="small", bufs=2, name="expA_p")
            nc.tensor.transpose(expA_p[:], expA_cols[:], ident[:])
            expA = cp.tile([H, 128], F32)
            nc.vector.tensor_copy(expA[:], expA_p[:])
            expB_p = ps.tile([H, 128], F32, tag="small", bufs=2, name="expB_p")
            nc.tensor.transpose(expB_p[:], expB_cols[:], ident[:])
            expB = cp.tile([H, 128], F32)
            nc.vector.tensor_copy(expB[:], expB_p[:])

            # ---- load inputs ----
            def load_from(view, shape, name):
                t = wpool.tile(shape, F32, tag=name, name=name)
                nc.sync.dma_start(out=t[:], in_=view)
                return t

            XT0 = load_from(bview(OFF_XT, D, N_NODES), [D, N_NODES], "xT")
            XT0R = wpool.tile([D, N_NODES], F32R, tag="xTr", name="xTr")
            nc.vector.tensor_copy(XT0R[:], XT0[:])
            EW12 = wpool.tile([128, N_HID], F32, tag="ew12", name="ew12")
            nc.sync.dma_start(out=EW12[0:64, :], in_=bview(OFF_EW1, 64, N_HID))
            nc.sync.dma_start(out=EW12[64:128, :], in_=bview(OFF_EW1, 64, N_HID))
            EW2P = load_from(bview(OFF_EW2P, 128, 1024), [128, 1024], "ew2p")
            WQ, WKS, WV, WE, BO, WO0, WO1 = [], [], [], [], [], [], []
            for i in range(2):
                base = OFF_W + i * LAYER_W
                WQ.append(load_from(bview(base, D, HD), [D, HD], f"wq{i}"))
                WKS.append(load_from(bview(base + 16384, D, HD), [D, HD], f"wks{i}"))
                WV.append(load_from(bview(base + 32768, D, HD), [D, HD], f"wv{i}"))
                WO0.append(load_from(bview(base + 49152, 128, D), [128, D], f"wo{i}a"))
                WO1.append(load_from(bview(base + 49152 + 8192, 128, D), [128, D], f"wo{i}b"))
                WE.append(load_from(bview(base + 65536, 1, H), [1, H], f"we{i}"))
                BO.append(load_from(bview(base + 65540, D, 1), [D, 1], f"bo{i}"))
            WQR, WKSR, WVR = [], [], []
            for i in range(2):
                for (lst, srcl, nm) in ((WQR, WQ, "wqr"), (WKSR, WKS, "wksr"),
                                        (WVR, WV, "wvr")):
                    t = wpool.tile([D, HD], F32R, tag=f"{nm}{i}", name=f"{nm}{i}")
                    nc.vector.tensor_copy(t[:], srcl[i][:])
                    lst.append(t)
            WPRJ = load_from(bview(OFF_W + 2 * LAYER_W, D, 1), [D, 1], "wproj")
            WPRJR = wpool.tile([D, 1], F32R, tag="wprjr", name="wprjr")
            nc.vector.tensor_copy(WPRJR[:], WPRJ[:])
            BPRJ = load_from(bview(OFF_W + 2 * LAYER_W + D, 1, 1), [1, 1], "bproj")

            def layer(li, XT):
                wq, wks, wv, we, bo = WQR[li], WKSR[li], WVR[li], WE[li], BO[li]
                wo0, wo1 = WO0[li], WO1[li]

                # ---- per-head Q/K (transposed [64 feat, 2176 nodes]) ----
                QTh = [wk.tile([D, N_NODES], F32R, tag=f"qth{h}", name=f"qth{h}")
                       for h in range(H)]
                KTh = [wk.tile([D, N_NODES], F32R, tag=f"kth{h}", name=f"kth{h}")
                       for h in range(H)]
                for h in range(H):
                    for ci, (c0, cw) in enumerate(CH2176):
                        qp = ps.tile([D, 512], F32, tag="qkv", bufs=2,
                                     name=f"q_p{h}_{ci}")
                        nc.tensor.matmul(qp[:, 0:cw], wq[:, 64 * h:64 * h + 64],
                                         XT[:, c0:c0 + cw])
                        kp = ps.tile([D, 512], F32, tag="qkv", bufs=2,
                                     name=f"k_p{h}_{ci}")
                        nc.tensor.matmul(kp[:, 0:cw], wks[:, 64 * h:64 * h + 64],
                                         XT[:, c0:c0 + cw])
                        nc.vector.tensor_copy(QTh[h][:, c0:c0 + cw], qp[:, 0:cw])
                        nc.scalar.copy(KTh[h][:, c0:c0 + cw], kp[:, 0:cw])
                # V node-major [128, 17*256]: V[p, 256*t+f] = V(node 128t+p, f)
                V = wk.tile([128, 17 * HD], F32R, tag="v", name="v")
                for t in range(17):
                    vp = ps.tile([128, HD], F32, tag="qkv", bufs=2, name=f"v_p{t}")
                    nc.tensor.matmul(vp[:], XT[:, 128 * t:128 * (t + 1)], wv[:])
                    if t % 2 == 0:
                        nc.vector.tensor_copy(V[:, HD * t:HD * (t + 1)], vp[:])
                    else:
                        nc.scalar.copy(V[:, HD * t:HD * (t + 1)], vp[:])

                # ---- W2[p, hh] = we[hh + 2*(p>=64)] ----
                web_p = ps.tile([128, H], F32, tag="small", bufs=2, name="web_p")
                nc.tensor.matmul(web_p[:], ones[:], we[:])
                web = wk.tile([128, H], F32, tag="web")
                nc.vector.tensor_copy(web[:], web_p[:])
                W2 = wk.tile([128, 2], F32, tag="w2")
                nc.vector.tensor_copy(W2[0:64, 0:1], web[0:64, 0:1])
                nc.vector.tensor_copy(W2[64:128, 0:1], web[64:128, 2:3])
                nc.vector.tensor_copy(W2[0:64, 1:2], web[0:64, 1:2])
                nc.vector.tensor_copy(W2[64:128, 1:2], web[64:128, 3:4])

                # ---- block-1 logits LA [128, 4096]: p=i+64*(h//2), f=(h%2)*2048+j
                # (tile is f32r so every producer emits PE-consumable rounded data)
                LA = wk.tile([128, 2 * N_HID], F32R, tag="la")
                LAf = LA[:].bitcast(F32)
                for hh in range(2):
                    for c4, (c0, cw) in enumerate(CH2048):
                        spt = ps.tile([64, 512], F32, tag="sph", bufs=3,
                                      name=f"s1t_{hh}_{c4}")
                        spb = ps.tile([64, 512], F32, tag="sph", bufs=3,
                                      name=f"s1b_{hh}_{c4}")
                        nc.tensor.matmul(spt[:, 0:cw],
                                         KTh[hh][:, IC], QTh[hh][:, c0:c0 + cw])
                        nc.tensor.matmul(spb[:, 0:cw],
                                         KTh[hh + 2][:, IC], QTh[hh + 2][:, c0:c0 + cw])
                        # LA = EW1*we + S  (fused)
                        nc.vector.scalar_tensor_tensor(
                            out=LA[0:64, 2048 * hh + c0:2048 * hh + c0 + cw],
                            in0=EW12[0:64, c0:c0 + cw], scalar=W2[0:64, hh:hh + 1],
                            in1=spt[:, 0:cw], op0=ALU.mult, op1=ALU.add)
                        nc.vector.scalar_tensor_tensor(
                            out=LA[64:128, 2048 * hh + c0:2048 * hh + c0 + cw],
                            in0=EW12[64:128, c0:c0 + cw], scalar=W2[64:128, hh:hh + 1],
                            in1=spb[:, 0:cw], op0=ALU.mult, op1=ALU.add)

                # ---- block-2 logits LB [128, 4096]: p=j%128, f=h*1024+(j//128)*64+o
                LB = wk.tile([128, 4096], F32R, tag="lb")
                LBf = LB[:].bitcast(F32)
                for h in range(H):
                    for cp8 in range(2):
                        sp2 = ps.tile([128, 512], F32, tag="sp", bufs=1,
                                      name=f"s2_{h}_{cp8}")
                        for c8 in range(8):
                            c = 8 * cp8 + c8
                            nc.tensor.matmul(
                                sp2[:, 64 * c8:64 * c8 + 64],
                                KTh[h][:, 128 * c:128 * (c + 1)], QTh[h][:, OC])
                        nc.vector.scalar_tensor_tensor(
                            out=LB[:, 1024 * h + 512 * cp8:1024 * h + 512 * (cp8 + 1)],
                            in0=EW2P[:, 512 * cp8:512 * (cp8 + 1)],
                            scalar=web[:, h:h + 1],
                            in1=sp2[:], op0=ALU.mult, op1=ALU.add)

                # ---- leaky relu (in place) ----
                nc.vector.scalar_tensor_tensor(out=LA[:], in0=LAf[:], scalar=0.2,
                                               in1=LAf[:], op0=ALU.mult, op1=ALU.max)
                nc.vector.scalar_tensor_tensor(out=LB[:], in0=LBf[:], scalar=0.2,
                                               in1=LBf[:], op0=ALU.mult, op1=ALU.max)

                # ---- per-partition (negated) row maxima ----
                nm1 = wk.tile([128, 2], F32, tag="nm1")
                nm2 = wk.tile([128, H], F32, tag="nm2")
                for hh in range(2):
                    nc.vector.tensor_reduce(
                        out=nm1[:, hh:hh + 1], in_=LAf[:, 2048 * hh:2048 * (hh + 1)],
                        op=ALU.max, axis=AX.X, negate=True)
                for h in range(H):
                    nc.vector.tensor_reduce(
                        out=nm2[:, h:h + 1], in_=LBf[:, 1024 * h:1024 * (h + 1)],
                        op=ALU.max, axis=AX.X, negate=True)

                # ---- global (negated) max -> row vector [1, 4] ----
                nm1T_p = ps.tile([2, 128], F32, tag="small", bufs=2, name="nm1T_p")
                nc.tensor.transpose(nm1T_p[:], nm1[:], ident[:])
                nm2T_p = ps.tile([H, 128], F32, tag="small", bufs=2, name="nm2T_p")
                nc.tensor.transpose(nm2T_p[:], nm2[:], ident[:])
                ra = wk.tile([2, 1], F32, tag="ra")
                rb = wk.tile([2, 1], F32, tag="rb")
                mb = wk.tile([H, 1], F32, tag="mb")
                nc.vector.tensor_reduce(out=ra[:], in_=nm1T_p[:, 0:64],
                                        op=ALU.min, axis=AX.X)
                nc.vector.tensor_reduce(out=rb[:], in_=nm1T_p[:, 64:128],
                                        op=ALU.min, axis=AX.X)
                nc.vector.tensor_reduce(out=mb[:], in_=nm2T_p[:], op=ALU.min, axis=AX.X)
                raT_p = ps.tile([1, 2], F32, tag="small", bufs=2, name="raT_p")
                nc.tensor.transpose(raT_p[:], ra[:], ident[0:2, 0:2])
                rbT_p = ps.tile([1, 2], F32, tag="small", bufs=2, name="rbT_p")
                nc.tensor.transpose(rbT_p[:], rb[:], ident[0:2, 0:2])
                mbT_p = ps.tile([1, H], F32, tag="small", bufs=2, name="mbT_p")
                nc.tensor.transpose(mbT_p[:], mb[:], ident[0:4, 0:4])
                nm1row = wk.tile([1, H], F32, tag="nm1row")
                nc.vector.tensor_copy(nm1row[:, 0:2], raT_p[:])
                nc.vector.tensor_copy(nm1row[:, 2:4], rbT_p[:])
                nmgrow = wk.tile([1, H], F32, tag="nmgrow")
                nc.vector.tensor_tensor(nmgrow[:], nm1row[:], mbT_p[:], op=ALU.min)

                # broadcast -m_g back to per-partition bias columns
                NMG1_p = ps.tile([128, 2], F32, tag="small", bufs=2, name="NMG1_p")
                nc.tensor.matmul(NMG1_p[0:64, :], ones[:, 0:64], nmgrow[:, 0:2])
                nc.tensor.matmul(NMG1_p[64:128, :], ones[:, 0:64], nmgrow[:, 2:4])
                NMG2_p = ps.tile([128, H], F32, tag="small", bufs=2, name="NMG2_p")
                nc.tensor.matmul(NMG2_p[:], ones[:], nmgrow[:])
                NMG1 = wk.tile([128, 2], F32, tag="nmg1")
                NMG2 = wk.tile([128, H], F32, tag="nmg2")
                nc.vector.tensor_copy(NMG1[:], NMG1_p[:])
                nc.vector.tensor_copy(NMG2[:], NMG2_p[:])

                # ---- U = exp(L - m_g) (written f32r in place), rowsums ----
                rs1 = wk.tile([128, 2], F32, tag="rs1")
                rs2 = wk.tile([128, H], F32, tag="rs2")
                for hh in range(2):
                    nc.scalar.activation(
                        LA[:, 2048 * hh:2048 * (hh + 1)],
                        LAf[:, 2048 * hh:2048 * (hh + 1)], AF.Exp,
                        bias=NMG1[:, hh:hh + 1], accum_out=rs1[:, hh:hh + 1])
                for h in range(H):
                    nc.scalar.activation(
                        LB[:, 1024 * h:1024 * (h + 1)],
                        LBf[:, 1024 * h:1024 * (h + 1)], AF.Exp,
                        bias=NMG2[:, h:h + 1], accum_out=rs2[:, h:h + 1])

                # ---- global denominators S_g [1, 4] ----
                s1a_p = ps.tile([1, 2], F32, tag="small", bufs=2, name="s1a_p")
                nc.tensor.matmul(s1a_p[:], half[:, 0:1], rs1[:])
                s1b_p = ps.tile([1, 2], F32, tag="small", bufs=2, name="s1b_p")
                nc.tensor.matmul(s1b_p[:], half[:, 1:2], rs1[:])
                s2r_p = ps.tile([1, H], F32, tag="small", bufs=2, name="s2r_p")
                nc.tensor.matmul(s2r_p[:], onesc[:], rs2[:])
                s1row = wk.tile([1, H], F32, tag="s1row")
                nc.vector.tensor_copy(s1row[:, 0:2], s1a_p[:])
                nc.vector.tensor_copy(s1row[:, 2:4], s1b_p[:])
                sgrow = wk.tile([1, H], F32, tag="sgrow")
                nc.vector.tensor_add(sgrow[:], s1row[:], s2r_p[:])
                sgcol_p = ps.tile([H, 1], F32, tag="small", bufs=2, name="sgcol_p")
                nc.tensor.transpose(sgcol_p[:], sgrow[:], ident[0:1, 0:1])
                sgcol = wk.tile([H, 1], F32, tag="sgcol")
                nc.vector.tensor_copy(sgcol[:], sgcol_p[:])
                sinv = wk.tile([H, 1], F32, tag="sinv")
                nc.vector.reciprocal(sinv[:], sgcol[:])
                # wo rows scaled by 1/S_g(head)
                rsc0_p = ps.tile([128, 1], F32, tag="small", bufs=2, name="rsc0_p")
                nc.tensor.matmul(rsc0_p[:], expA[:], sinv[:])
                rsc1_p = ps.tile([128, 1], F32, tag="small", bufs=2, name="rsc1_p")
                nc.tensor.matmul(rsc1_p[:], expB[:], sinv[:])
                rsc0 = wk.tile([128, 1], F32, tag="rsc0")
                rsc1 = wk.tile([128, 1], F32, tag="rsc1")
                nc.vector.tensor_copy(rsc0[:], rsc0_p[:])
                nc.vector.tensor_copy(rsc1[:], rsc1_p[:])
                wos0 = wk.tile([128, D], F32R, tag="wos0")
                wos1 = wk.tile([128, D], F32R, tag="wos1")
                nc.vector.tensor_scalar_mul(wos0[:], wo0[:], rsc0[:])
                nc.vector.tensor_scalar_mul(wos1[:], wo1[:], rsc1[:])

                # ---- aggregation ----
                # V_input duplicated on both partition halves
                VI2 = wk.tile([128, HD], F32R, tag="vi2")
                nc.vector.tensor_copy(VI2[0:64, :], V[0:64, 16 * HD:17 * HD])
                nc.vector.tensor_copy(VI2[64:128, :], V[0:64, 16 * HD:17 * HD])
                # block1: A1S[pair][64*(h%2)+d, j] = sum_i Vhat_I[i, hd] * U1[i, hj]
                A1S = [wk.tile([128, N_HID], F32R, tag=f"a1s{i}", name=f"a1s{i}")
                       for i in range(2)]
                for pair in range(2):
                    pb = 64 * pair  # LA/VI2 partition base for this head pair
                    for c4, (c0, cw) in enumerate(CH2048):
                        ap0 = ps.tile([64, 512], F32, tag="sph", bufs=3,
                                      name=f"a1p{pair}_{c4}a")
                        ap1 = ps.tile([64, 512], F32, tag="sph", bufs=3,
                                      name=f"a1p{pair}_{c4}b")
                        for hh, ap in ((0, ap0), (1, ap1)):
                            h = hh + 2 * pair
                            nc.tensor.matmul(
                                ap[:, 0:cw],
                                VI2[pb:pb + 64, 64 * h:64 * h + 64],
                                LA[pb:pb + 64, 2048 * hh + c0:2048 * hh + c0 + cw])
                        if c4 % 2 == 0:
                            nc.vector.tensor_copy(A1S[pair][0:64, c0:c0 + cw], ap0[:, 0:cw])
                            nc.vector.tensor_copy(A1S[pair][64:128, c0:c0 + cw], ap1[:, 0:cw])
                        else:
                            nc.scalar.copy(A1S[pair][0:64, c0:c0 + cw], ap0[:, 0:cw])
                            nc.scalar.copy(A1S[pair][64:128, c0:c0 + cw], ap1[:, 0:cw])
                # block2: A2S[pair][64*(h%2)+d, o] = sum_j Vhat[j, hd] * U2[j, ho]
                A2S = [wk.tile([128, N_OUT], F32R, tag=f"a2s{i}", name=f"a2s{i}")
                       for i in range(2)]
                for pair in range(2):
                    for hh in range(2):
                        h = hh + 2 * pair
                        a2p = ps.tile([64, N_OUT], F32, tag="small", bufs=2,
                                      name=f"a2p{pair}_{hh}")
                        for c in range(16):
                            nc.tensor.matmul(
                                a2p[:],
                                V[:, HD * c + 64 * h:HD * c + 64 * h + 64],
                                LB[:, 1024 * h + 64 * c:1024 * h + 64 * c + 64],
                                start=(c == 0), stop=(c == 15))
                        nc.vector.tensor_copy(
                            A2S[pair][64 * hh:64 * hh + 64, :], a2p[:])

                # ---- output projection + residual + relu ----
                X2 = wk.tile([D, N_NODES], F32R, tag="x2", bufs=2)
                for c4, (c0, cw) in enumerate(CH2048):
                    xcp = ps.tile([D, 512], F32, tag="sp", bufs=1, name=f"xcp{c4}")
                    nc.tensor.matmul(xcp[:, 0:cw], wos0[:], A1S[0][:, c0:c0 + cw],
                                     start=True, stop=False)
                    nc.tensor.matmul(xcp[:, 0:cw], wos1[:], A1S[1][:, c0:c0 + cw],
                                     start=False, stop=True)
                    tC = wk.tile([D, 512], F32, tag="tc_res", bufs=2)
                    nc.vector.tensor_add(tC[:, 0:cw], xcp[:, 0:cw], XT[:, c0:c0 + cw].bitcast(F32))
                    nc.scalar.activation(X2[:, c0:c0 + cw], tC[:, 0:cw],
                                         AF.Relu, bias=bo[:])
                xop = ps.tile([D, N_OUT], F32, tag="small", bufs=2, name="xop")
                nc.tensor.matmul(xop[:], wos0[:], A2S[0][:], start=True, stop=False)
                nc.tensor.matmul(xop[:], wos1[:], A2S[1][:], start=False, stop=True)
                tO = wk.tile([D, N_OUT], F32, tag="to_res")
                nc.vector.tensor_add(tO[:], xop[:], XT[:, OC].bitcast(F32))
                nc.scalar.activation(X2[:, OC], tO[:], AF.Relu, bias=bo[:])
                nc.scalar.activation(X2[:, IC], XT[:, IC].bitcast(F32), AF.Relu, bias=bo[:])
                return X2

            def final(X3):
                prj_p = ps.tile([N_OUT, 1], F32, tag="small", bufs=2, name="prj_p")
                nc.tensor.matmul(prj_p[:], X3[:, OC].bitcast(F32), WPRJ[:])
                bpb_p = ps.tile([N_OUT, 1], F32, tag="small", bufs=2, name="bpb_p")
                nc.tensor.matmul(bpb_p[:], ones[:, 0:64], BPRJ[:])
                bpb = wk.tile([N_OUT, 1], F32, tag="bpb")
                nc.vector.tensor_copy(bpb[:], bpb_p[:])
                ovec = wk.tile([N_OUT, 1], F32, tag="ovec")
                nc.scalar.activation(ovec[:], prj_p[:],
                                     mybir.ActivationFunctionType.Sigmoid,
                                     bias=bpb[:])
                nc.sync.dma_start(out=xout_d[:, N_NODES:N_NODES + 1], in_=ovec[:])
                nc.sync.dma_start(out=xout_d[:, 0:N_NODES], in_=X3[:].bitcast(F32))

            loop_n = int(os.environ.get("GNN_LOOP", "1"))
            if loop_n > 1:
                with tc.For_i(0, loop_n, 1):
                    X2 = layer(0, XT0R)
                    X3 = layer(1, X2)
                    final(X3)
            else:
                X2 = layer(0, XT0R)
                X3 = layer(1, X2)
                final(X3)

    nc.compile()
    return nc


# ---------------------------------------------------------------------------
# Host-side packing / runner
# ---------------------------------------------------------------------------

def _pack_inputs(inputs):
    x = np.asarray(inputs["node_features"], np.float32).copy()
    x[:N_IN, 0] = np.asarray(inputs["x_input"], np.float32)
    xT = np.ascontiguousarray(x.T)  # [64, 2176] natural order
    # reorder cols to [hidden | input | output]
    xTr = np.concatenate([xT[:, N_IN:N_IN + N_HID], xT[:, :N_IN],
                          xT[:, N_IN + N_HID:]], axis=1)
    ew = np.asarray(inputs["edge_weights"], np.float32).reshape(-1)
    ew1 = ew[:E1].reshape(N_IN, N_HID)
    ew2 = ew[E1:].reshape(N_HID, N_OUT)
    # EW2P[p, c*64+o] = ew2[c*128+p, o]
    ew2p = ew2.reshape(16, 128, N_OUT).transpose(1, 0, 2).reshape(128, 1024)

    parts = [xTr.ravel(), ew1.ravel(), ew2p.ravel()]
    for l in (1, 2):
        parts += [
            np.asarray(inputs[f"wq{l}"], np.float32).ravel(),
            (np.asarray(inputs[f"wk{l}"], np.float32) * 0.125).ravel(),
            np.asarray(inputs[f"wv{l}"], np.float32).ravel(),
            np.asarray(inputs[f"wo{l}"], np.float32).ravel(),
            np.asarray(inputs[f"we{l}"], np.float32).ravel(),
            np.asarray(inputs[f"bo{l}"], np.float32).ravel(),
        ]
    parts += [np.asarray(inputs["wproj"], np.float32).ravel(),
              np.asarray(inputs["bproj"], np.float32).ravel()]
    blob = np.concatenate(parts)[None, :].astype(np.float32)
    assert blob.shape[1] == NBLOB, blob.shape
    return blob


def _unpack(xout):
    # xout [64, 2177]: cols 0:2048 hidden, 2048:2112 input, 2112:2176 output,
    # 2176 = sigmoid output vector
    x_full = np.empty((N_NODES, D), np.float32)
    x_full[N_IN:N_IN + N_HID] = xout[:, 0:N_HID].T
    x_full[:N_IN] = xout[:, N_HID:N_HID + N_IN].T
    x_full[N_IN + N_HID:] = xout[:, N_HID + N_IN:N_NODES].T
    out = xout[:, N_NODES].reshape(N_OUT).astype(np.float32)
    return out, x_full


@functools.lru_cache(maxsize=1)
def _get_runner():
    """Build + compile the program once; return callable(blob) -> xout."""
    import jax
    import concourse.mybir as mybir
    from concourse.bass2jax import (_bass_exec_p, install_neuronx_cc_hook,
                                    partition_id_tensor)

    nc = _build_program()
    install_neuronx_cc_hook()

    partition_name = nc.partition_id_tensor.name if nc.partition_id_tensor else None
    in_names, out_names, out_avals, out_shapes = [], [], [], []
    for alloc in nc.m.functions[0].allocations:
        if not isinstance(alloc, mybir.MemoryLocationSet):
            continue
        name = alloc.memorylocations[0].name
        if alloc.kind == "ExternalInput":
            if name != partition_name:
                in_names.append(name)
        elif alloc.kind == "ExternalOutput":
            out_names.append(name)
            shape = tuple(alloc.tensor_shape)
            dtype = mybir.dt.np(alloc.dtype)
            out_avals.append(jax.core.ShapedArray(shape, dtype))
            out_shapes.append((shape, dtype))
    assert in_names == ["blob"] and out_names == ["xout"], (in_names, out_names)
    in_names_all = list(in_names) + out_names + (
        [partition_name] if partition_name else [])

    def _body(*args):
        operands = list(args)
        if partition_name is not None:
            operands.append(partition_id_tensor())
        outs = _bass_exec_p.bind(
            *operands, out_avals=tuple(out_avals), in_names=tuple(in_names_all),
            out_names=tuple(out_names), lowering_input_output_aliases=(),
            sim_require_finite=False, sim_require_nnan=False, nc=nc)
        return tuple(outs)

    jitted = jax.jit(_body, donate_argnums=(1,), keep_unused=True)
    xout_shape = out_shapes[0][0]

    def run(blob):
        zeros = np.zeros(xout_shape, np.float32)
        out_arrs = jitted(blob, zeros)
        return np.asarray(out_arrs[0])

    return run


# ---------------------------------------------------------------------------
# General fallback (arbitrary edge_index) — exact reference math via jax
# ---------------------------------------------------------------------------

def _fallback(inputs):
    import jax
    import jax.numpy as jnp

    def message_pass(x, edge_index, ew, wq, wk, wv, we, wo, bo):
        src, tgt = edge_index[0], edge_index[1]
        dout = wo.shape[1]
        heads = wq.shape[1] // dout
        q = (x[tgt] @ wq).reshape(-1, heads, dout)
        k = (x[src] @ wk).reshape(-1, heads, dout)
        v = (x[src] @ wv).reshape(-1, heads, dout)
        attn = (q * k).sum(-1) / jnp.sqrt(jnp.float32(dout)) + ew @ we
        attn = jax.nn.leaky_relu(attn, negative_slope=0.2)
        attn = jax.nn.softmax(attn, axis=0)
        weighted_v = (attn[:, :, None] * v).reshape(-1, heads * dout)
        out = jax.ops.segment_sum(weighted_v, tgt, num_segments=x.shape[0])
        return out @ wo + bo + x

    f = inputs
    x = jnp.asarray(f["node_features"], jnp.float32)
    x = x.at[:jnp.asarray(f["x_input"]).shape[0], 0].set(jnp.asarray(f["x_input"]))
    ei = jnp.asarray(f["edge_index"], jnp.int32)
    ew = jnp.asarray(f["edge_weights"], jnp.float32)
    x = jax.nn.relu(message_pass(x, ei, ew, f["wq1"], f["wk1"], f["wv1"],
                                 f["we1"], f["wo1"], f["bo1"]))
    x = jax.nn.relu(message_pass(x, ei, ew, f["wq2"], f["wk2"], f["wv2"],
                                 f["we2"], f["wo2"], f["bo2"]))
    n_out = int(f["num_output_nodes"])
    out_nodes = x[x.shape[0] - n_out:]
    output = jax.nn.sigmoid(out_nodes @ jnp.asarray(f["wproj"])
                            + jnp.asarray(f["bproj"])).squeeze()
    return np.asarray(output, np.float32), np.asarray(x, np.float32)


# ---------------------------------------------------------------------------

def _is_structured(inputs):
    try:
        ei = np.asarray(inputs["edge_index"])
        if ei.shape != (2, E):
            return False
        if int(inputs["num_output_nodes"]) != N_OUT:
            return False
        if np.asarray(inputs["node_features"]).shape != (N_NODES, D):
            return False
        return bool(np.array_equal(ei, _expected_edge_index()))
    except Exception:
        return False


def kernel(**inputs):
    if not _is_structured(inputs):
        return _fallback(inputs)
    run = _get_runner()
    return _unpack(run(_pack_inputs(inputs)))
